# revision 1
# baseline (speedup 1.0000x reference)
"""HMM loss kernel for Trainium2 (8 NeuronCores, vocab-parallel).

Problem shapes (hardcoded): B,T,K,LS = 4,8,4,4; PH=B*T*K=128, TL=32,
H=512, V=32000, NS=128.

The dominant cost of the reference is the generator projection
[PH*TL, H] @ [H, V] plus its log-softmax.  Two observations make this
cheap:

1. Only tokens inside the inclusive span [tgt_idx[p,0], tgt_idx[p,1]]
   contribute to the loss (everything else is masked to 0 before the
   span-sum), so only ~1/3 of the 4096 tokens need logits at all.
2. log_softmax is only consumed via psk = logit[target] - logsumexp(logits).
   logit[target] is a single f32 dot product per token; logsumexp needs the
   full vocab row, which we shard over the 8 cores (4000 columns each) and
   combine on the host with a single log.

Device work per core: fp8 DoubleRow matmul [n_act,512]x[512,4000] into PSUM
(f32), ScalarE exp (with the fp8 scaling folded into the activation scale)
accumulating the partial sum-of-exp along the vocab axis, and an f32
elementwise dot for this core's slice of target logits.  The fp8 noise only
touches logsumexp (~1e-4 absolute); target logits stay f32.  Everything
after that (span sums, chain scores, the T=8/K=4 backward scan) is a few
thousand flops and runs on the host.
"""

import math
from contextlib import ExitStack

import ml_dtypes
import numpy as np

B, T, K, LS = 4, 8, 4, 4
PH, TL, H, V, NS = B * T * K, 32, 512, 32000, 128
NCORES = 8
VS = V // NCORES  # vocab shard per core
VT_WIDTHS = [512] * (VS // 512) + ([VS % 512] if VS % 512 else [])
HC = H // 128  # contraction chunks
XSCALE = 16.0  # fp8 pre-scales keep W (sigma~0.02) out of e4m3 subnormals
WSCALE = 256.0
FP8 = True


def _split_sync_waits(nc, maxw=1):
    """This container's walrus rejects instructions carrying more than a
    couple of sync-wait commands, while Tile freely attaches one wait per
    dependency.  Hoist excess waits onto standalone EventSemaphore
    instructions inserted just before the owner on the same engine queue."""
    import concourse.mybir as mybir

    ctr = 0
    for fn in nc.m.functions:
        for bb in fn.blocks:
            out = []
            changed = False
            for inst in bb.instructions:
                si = getattr(inst, "sync_info", None)
                waits = list(si.on_wait) if si is not None and si.on_wait else []
                if len(waits) > maxw:
                    changed = True
                    extra, keep = waits[:-maxw], waits[-maxw:]
                    for i in range(0, len(extra), maxw):
                        ctr += 1
                        out.append(
                            mybir.InstEventSemaphore(
                                name=f"W-split-{ctr}",
                                engine=inst.engine,
                                ins=[],
                                outs=[],
                                sync_info=mybir.SyncInfo(
                                    on_wait=extra[i : i + maxw], on_update=[]
                                ),
                            )
                        )
                    inst.sync_info = mybir.SyncInfo(
                        on_wait=keep, on_update=list(si.on_update or [])
                    )
                out.append(inst)
            if changed:
                bb.instructions = out


_BUILD_CACHE = {}


def _build(n_chunks, ntc, with_bias, repeat=1, fp8=FP8, group=4):
    """Build the per-core bass program.

    n_chunks: number of 128-token chunks of active tokens (all cores see all
    of them for the sum-exp partials).
    ntc: chunks of target-logit dot products handled by each core.
    repeat: emit the whole body (including input DMAs) this many times —
    used only by the timing harness to measure marginal per-pass HW time.
    group: PSUM banks per ACTIVATE (1/2/4) — larger groups amortize the
    ~352-cycle ACT instruction overhead, smaller ones pipeline better.
    """
    key = (n_chunks, ntc, with_bias, repeat, fp8, group)
    if key in _BUILD_CACHE:
        return _BUILD_CACHE[key]

    import concourse.bass as bass
    import concourse.mybir as mybir
    import concourse.tile as tile

    n_pad = n_chunks * 128
    f8 = mybir.dt.float8e4
    bf16 = mybir.dt.bfloat16
    f32 = mybir.dt.float32
    mmdt = f8 if fp8 else bf16
    exp_scale = float(1.0 / (XSCALE * WSCALE)) if fp8 else 1.0

    nc = bass.Bass()
    # [partition, k-subtile, col] layout: element (p, s, c) = row s*128+p
    xt_d = nc.dram_tensor("xt", [128, HC, n_pad], mmdt, kind="ExternalInput")
    w_d = nc.dram_tensor("w", [128, HC, VS], mmdt, kind="ExternalInput")
    xr_d = nc.dram_tensor("xr", [ntc, 128, H], f32, kind="ExternalInput")
    wt_d = nc.dram_tensor("wt", [ntc, 128, H], f32, kind="ExternalInput")
    if with_bias:
        bb_d = nc.dram_tensor("bb", [1, VS], bf16, kind="ExternalInput")
    se_d = nc.dram_tensor("se", [128, n_chunks], f32, kind="ExternalOutput")
    tl_d = nc.dram_tensor("tl", [128, ntc], f32, kind="ExternalOutput")

    with tile.TileContext(nc) as tc, ExitStack() as ctx:
        consts = ctx.enter_context(tc.tile_pool(name="consts", bufs=2))
        psum = ctx.enter_context(
            tc.tile_pool(name="psum", bufs=8 // group, space="PSUM")
        )
        acc = ctx.enter_context(tc.tile_pool(name="acc", bufs=1))
        outp = ctx.enter_context(tc.tile_pool(name="outp", bufs=1))
        dots = ctx.enter_context(tc.tile_pool(name="dots", bufs=2))

        for _rep in range(repeat):
            xt_sb = consts.tile([128, HC, n_pad], mmdt, tag="xt")
            nc.sync.dma_start(out=xt_sb, in_=xt_d[:, :, :])
            w_sb = []
            for vt, vtw in enumerate(VT_WIDTHS):
                voff = vt * 512
                wtile = consts.tile([128, HC, 512], mmdt, tag=f"wv{vt}")
                nc.sync.dma_start(
                    out=wtile[:, :, :vtw], in_=w_d[:, :, voff : voff + vtw]
                )
                w_sb.append(wtile)
            if with_bias:
                ones_sb = consts.tile([1, 128], bf16, tag="ones")
                nc.vector.memset(ones_sb, 1.0)
                b_sb = consts.tile([1, VS], bf16, tag="bias")
                nc.sync.dma_start(out=b_sb, in_=bb_d[0:1, :])

            se_all = outp.tile([128, n_chunks], f32, tag="se")
            tl_all = outp.tile([128, ntc], f32, tag="tl")

            # target-logit dots up front: DVE and the DMA queues are idle
            # while the matmul pipeline warms, and this keeps them off the
            # kernel tail
            for c in range(ntc):
                xr_sb = dots.tile([128, H], f32, tag="xr")
                nc.sync.dma_start(out=xr_sb, in_=xr_d[c])
                wt_sb = dots.tile([128, H], f32, tag="wt")
                nc.sync.dma_start(out=wt_sb, in_=wt_d[c])
                prod = dots.tile([128, H], f32, tag="prod")
                nc.vector.tensor_mul(prod, xr_sb, wt_sb)
                nc.vector.reduce_sum(
                    out=tl_all[:, c : c + 1], in_=prod, axis=mybir.AxisListType.X
                )
            nc.sync.dma_start(out=tl_d[:, :], in_=tl_all)

            groups = []  # (vt_list, act_width)
            for g0 in range(0, len(VT_WIDTHS), group):
                vts = list(range(g0, min(g0 + group, len(VT_WIDTHS))))
                gw = 512 * (len(vts) - 1) + VT_WIDTHS[vts[-1]]
                groups.append((vts, gw))

            for tci in range(n_chunks):
                splits = 1
                separt = acc.tile([128, len(groups) * group], f32, tag="sep",
                                  bufs=3, name="sep")
                nsep = 0
                for gi, (vts, gw) in enumerate(groups):
                    ps = psum.tile([128, 512 * group], f32)
                    for slot, vt in enumerate(vts):
                        vtw = VT_WIDTHS[vt]
                        voff = vt * 512
                        pslice = ps[:, slot * 512 : slot * 512 + vtw]
                        if fp8:
                            for s in range(0, HC, 2):
                                nc.tensor.matmul(
                                    pslice,
                                    lhsT=xt_sb[:, s : s + 2,
                                               tci * 128 : (tci + 1) * 128],
                                    rhs=w_sb[vt][:, s : s + 2, :vtw],
                                    start=(s == 0),
                                    stop=(s == HC - 2) and not with_bias,
                                    perf_mode=mybir.MatmulPerfMode.DoubleRow,
                                )
                        else:
                            for s in range(HC):
                                nc.tensor.matmul(
                                    pslice,
                                    lhsT=xt_sb[:, s, tci * 128 : (tci + 1) * 128],
                                    rhs=w_sb[vt][:, s, :vtw],
                                    start=(s == 0),
                                    stop=(s == HC - 1) and not with_bias,
                                )
                        if with_bias:
                            nc.tensor.matmul(
                                pslice,
                                lhsT=ones_sb,
                                rhs=b_sb[:, voff : voff + vtw],
                                start=False,
                                stop=True,
                            )
                    if splits == 1:
                        nc.scalar.activation(
                            out=ps[:, :gw],
                            in_=ps[:, :gw],
                            func=mybir.ActivationFunctionType.Exp,
                            scale=exp_scale,
                            accum_out=separt[:, nsep : nsep + 1],
                        )
                        nsep += 1
                    else:
                        for slot, vt in enumerate(vts):
                            lo = slot * 512
                            hi = lo + VT_WIDTHS[vt]
                            nc.scalar.activation(
                                out=ps[:, lo:hi],
                                in_=ps[:, lo:hi],
                                func=mybir.ActivationFunctionType.Exp,
                                scale=exp_scale,
                                accum_out=separt[:, nsep : nsep + 1],
                            )
                            nsep += 1
                nc.vector.reduce_sum(
                    out=se_all[:, tci : tci + 1], in_=separt[:, :nsep],
                    axis=mybir.AxisListType.X,
                )

            nc.sync.dma_start(out=se_d[:, :], in_=se_all)

    _split_sync_waits(nc)
    _BUILD_CACHE[key] = nc
    return nc


def _prep_inputs(output, W, b, target, tgt_idx, fp8=FP8):
    """Host-side sharding/layout prep. Returns (in_maps, meta)."""
    x = np.asarray(output, np.float32).reshape(PH * TL, H)
    tgt = np.asarray(target, np.int32).reshape(-1)
    ti = np.asarray(tgt_idx, np.int32)
    bv = np.asarray(b, np.float32).reshape(-1)
    with_bias = bool(np.any(bv != 0.0))

    pos = np.arange(TL)
    span = (pos[None, :] >= ti[:, :1]) & (pos[None, :] <= ti[:, 1:2])
    act = np.flatnonzero(span.reshape(-1))
    n_act = int(act.size)
    n_chunks = max(1, math.ceil(n_act / 128))
    n_pad = n_chunks * 128
    act_pad = np.zeros(n_pad, np.int64)
    act_pad[:n_act] = act

    Wf = np.asarray(W, np.float32)
    xa = x[act_pad]  # [n_pad, H] f32

    if fp8:
        mmnp = ml_dtypes.float8_e4m3
        x_m = (xa * XSCALE).astype(mmnp)
        w_m = (Wf * WSCALE).astype(mmnp)
    else:
        mmnp = ml_dtypes.bfloat16
        x_m = xa.astype(mmnp)
        w_m = Wf.astype(mmnp)

    # [H, n_pad] -> [HC,128,n_pad] -> [128,HC,n_pad] (partition, k-subtile, col)
    xt = np.ascontiguousarray(
        x_m.T.reshape(HC, 128, n_pad).transpose(1, 0, 2)
    )

    wtT = Wf.T[tgt[act_pad]]  # [n_pad, H] f32 gathered target columns

    ntc = max(1, math.ceil(n_chunks / NCORES))
    per = ntc * 128
    xr_all = np.zeros((NCORES, ntc, 128, H), np.float32)
    wt_all = np.zeros((NCORES, ntc, 128, H), np.float32)
    for i in range(NCORES):
        lo = i * per
        hi = min(lo + per, n_pad)
        if hi > lo:
            blk = np.zeros((per, H), np.float32)
            blk[: hi - lo] = xa[lo:hi]
            xr_all[i] = blk.reshape(ntc, 128, H)
            blk = np.zeros((per, H), np.float32)
            blk[: hi - lo] = wtT[lo:hi]
            wt_all[i] = blk.reshape(ntc, 128, H)

    in_maps = []
    for i in range(NCORES):
        wsh = np.ascontiguousarray(
            w_m[:, i * VS : (i + 1) * VS].reshape(HC, 128, VS).transpose(1, 0, 2)
        )
        m = {"xt": xt, "w": wsh, "xr": xr_all[i], "wt": wt_all[i]}
        if with_bias:
            m["bb"] = bv[i * VS : (i + 1) * VS].astype(ml_dtypes.bfloat16).reshape(1, VS)
        in_maps.append(m)

    meta = dict(
        act=act, act_pad=act_pad, n_act=n_act, n_chunks=n_chunks, n_pad=n_pad,
        ntc=ntc, tgt=tgt, with_bias=with_bias, bv=bv, fp8=fp8,
    )
    return in_maps, meta


def _combine(results, meta):
    """Host-side unshard: total sum-exp across vocab shards -> psk."""
    n_act, n_pad, ntc = meta["n_act"], meta["n_pad"], meta["ntc"]
    se = np.zeros((128, meta["n_chunks"]), np.float64)
    for r in results:
        se += r["se"].astype(np.float64)
    sumexp_tok = se.T.reshape(-1)  # token t = chunk*128 + lane
    tl_tok = np.concatenate([r["tl"].T.reshape(-1) for r in results])[:n_pad]
    tl_tok = tl_tok.astype(np.float64)
    if meta["with_bias"]:
        tl_tok = tl_tok + meta["bv"][meta["tgt"][meta["act_pad"]]]

    logz = np.log(sumexp_tok[:n_act])
    psk_act = tl_tok[:n_act] - logz
    psk = np.zeros(PH * TL, np.float64)
    psk[meta["act"]] = psk_act
    return psk.reshape(PH, TL)


def _hmm_tail(psk, tgt_idx, states, init_logps, trans_logps, ext_logps, hsmm_sid):
    """Direct numpy port of the reference below the log-softmax."""
    ti = np.asarray(tgt_idx, np.int32)
    st4 = np.asarray(states, np.int64)
    init_logps = np.asarray(init_logps, np.float64)
    trans_logps = np.asarray(trans_logps, np.float64)
    ext_logps = np.asarray(ext_logps, np.float64)
    sid = int(np.asarray(hsmm_sid))

    pos = np.arange(TL)
    span = (pos[None, :] >= ti[:, :1]) & (pos[None, :] <= ti[:, 1:2])
    fwd_obs = np.where(span, psk, 0.0).sum(axis=1)  # [PH]

    st = st4.reshape(PH, LS)
    chain = trans_logps[st[:, :-1], st[:, 1:]].sum(axis=1)  # [PH]
    init_pmt = (init_logps[st[:, 0]] + chain).reshape(B, T, K)
    pmt = chain.reshape(B, T, K)
    obs = fwd_obs.reshape(B, T, K)
    z = np.where((np.arange(T) == 0)[None, :, None], init_pmt, pmt)
    s_first = st4[..., 0]  # [B,T,K]
    s_last = st4[..., -1]
    ov = np.any(
        st4[:, :-1, :, None, :, None] == st4[:, 1:, None, :, None, :], axis=(-1, -2)
    )  # [B,T-1,K,K]

    def lse2(x):  # logsumexp over last axis, -inf safe
        m = np.max(x, axis=-1, keepdims=True)
        ms = np.where(np.isfinite(m), m, 0.0)
        with np.errstate(divide="ignore"):
            return np.log(np.exp(x - ms).sum(axis=-1)) + ms[..., 0]

    beta = np.zeros((B, K), np.float64)
    for t in range(T - 2, -1, -1):
        sl = s_last[:, t]
        sf = s_first[:, t + 1]
        tr = (
            trans_logps[sl[:, :, None], sf[:, None, :]]
            + ext_logps[sl[:, :, None], sf[:, None, :]]
        )
        score = (
            beta[:, None, :]
            + obs[:, t + 1][:, None, :]
            + z[:, t + 1][:, None, :]
            + z[:, t][:, :, None]
            + tr
        )
        if K > 1:
            score = np.where(ov[:, t], -np.inf, score)
        beta = lse2(score)

    score0 = beta + obs[:, 0] + z[:, 0] + ext_logps[sid, s_first[:, 0]]
    log_marg = lse2(score0)
    return -np.sum(log_marg)


def kernel(output, W, b, target, tgt_idx, states, init_logps, trans_logps,
           ext_logps, hsmm_sid):
    from concourse.bass_utils import run_bass_kernel_spmd

    in_maps, meta = _prep_inputs(output, W, b, target, tgt_idx)
    nc = _build(meta["n_chunks"], meta["ntc"], meta["with_bias"])
    last_err = None
    for _attempt in range(3):
        try:
            res = run_bass_kernel_spmd(nc, in_maps, core_ids=list(range(NCORES)))
            break
        except Exception as e:  # rare transient device-unrecoverable flakes
            last_err = e
            import time as _time

            _time.sleep(2.0)
    else:
        raise last_err
    psk = _combine(res.results, meta)
    loss = _hmm_tail(psk, tgt_idx, states, init_logps, trans_logps, ext_logps, hsmm_sid)
    return np.float32(loss)

